# revision 23
# baseline (speedup 1.0000x reference)
"""Trainium2 Bass kernel for a single decoder layer (MHA + residual).

Sharding: 8 cores = 2 batches x 4 head-groups. Core c handles batch c//4
and heads 4*(c%4) .. 4*(c%4)+3 (tensor-parallel over heads; the W_o
all-reduce is done host-side by summing the 4 partial outputs per batch).

Host passes inputs pre-arranged so every DMA reads large contiguous
per-partition blocks (measured ~10x faster than 2KB-strided patterns).

Hardcoded for: X [2, 2048, 1024], W_* [1024, 1024], 16 heads, d_k=64.
"""

import sys

for _p in ("/root/.axon_site/_ro/trn_rl_repo", "/opt/trn_rl_repo"):
    if _p not in sys.path:
        sys.path.append(_p)

import numpy as np

import concourse.bacc as bacc
import concourse.mybir as mybir
import concourse.tile as tile
from concourse.bass_utils import run_bass_kernel_spmd

P = 128
S = 2048          # per-core sequence length (one batch)
D = 1024
NH = 4            # heads per core
HP = 2            # head pairs per core
DK = 64
CH = 512          # seq chunk (free dim for projections / q tile)
NCH = S // CH     # 4
KT = D // P       # 8 contraction tiles for the projections
NB = S // P       # 16 key blocks
F32 = mybir.dt.float32
F32R = mybir.dt.float32r
EXP = mybir.ActivationFunctionType.Exp

_CACHE = {}


import os
PHASE = os.environ.get("KPHASE", "all")


def _emit(nc, tc, pools, tiles, dram):
    const, xtp, vtp, ep, rp, ps1, sps, pvs = pools
    (wq_sb, wk_sb, wv_sb, wo_sb, pos_sb, msk_sb, id_sb, ones1,
     QT, KTt, AT, VT) = tiles
    xt_d, wqt_d, wkt_d, wvt_d, wot_d, post_d, dmask_d, ident_d, part_d = dram

    xt_tiles = [xtp.tile([P, KT, CH], F32R, tag="xt", name=f"xt_c{c}")
                for c in range(NCH)]
    # chunk0 split in two so the first matmuls start after ~1MB of DMA
    nc.sync.dma_start(xt_tiles[0][:, 0:4, :], xt_d[:, 0, 0:4, :])
    nc.sync.dma_start(wq_sb, wqt_d[:, :, :])
    nc.sync.dma_start(xt_tiles[0][:, 4:8, :], xt_d[:, 0, 4:8, :])
    nc.sync.dma_start(wk_sb, wkt_d[:, :, :])
    nc.sync.dma_start(wv_sb, wvt_d[:, :, :])
    nc.sync.dma_start(pos_sb, post_d[:, :])
    nc.sync.dma_start(id_sb, ident_d[:, :])
    nc.sync.dma_start(msk_sb, dmask_d[:, :, :, :])
    nc.sync.dma_start(wo_sb, wot_d[:, :, :])

    if PHASE == "dmain":
        for c in range(NCH):
            if c > 0:
                nc.sync.dma_start(xt_tiles[c], xt_d[:, c, :, :])
            ob = vtp.tile([P, 2, CH], F32, tag="ob", name="ob")
            nc.vector.tensor_copy(ob[:, 0, :], xt_tiles[c][:, 0, :].bitcast(F32))
            nc.sync.dma_start(part_d[c * P:(c + 1) * P, 0:CH], ob[:, 0, :])
        return

    # ---- projections, chunk by chunk ----
    for c in range(NCH):
        xt_c = xt_tiles[c]
        if c > 0:
            nc.sync.dma_start(xt_c, xt_d[:, c, :, :])
        for p in range(HP if PHASE != "projq" else 1):
            mcol = p * P
            psq = ps1.tile([P, CH], F32, tag="ps1", name="psq")
            for k in range(KT):
                nc.tensor.matmul(psq, wq_sb[:, k, mcol:mcol + P], xt_c[:, k, :],
                                 start=(k == 0), stop=(k == KT - 1))
            nc.vector.tensor_add(QT[p][c], psq, pos_sb[:, c * CH:(c + 1) * CH])

            if PHASE == "projq":
                continue
            psk = ps1.tile([P, CH], F32, tag="ps1", name="psk")
            for k in range(KT):
                nc.tensor.matmul(psk, wk_sb[:, k, mcol:mcol + P], xt_c[:, k, :],
                                 start=(k == 0), stop=(k == KT - 1))
            nc.vector.tensor_add(KTt[p][c], psk, pos_sb[:, c * CH:(c + 1) * CH])

            # V^T then PE-transpose into natural [s, head, dk] layout.
            psv = ps1.tile([P, CH], F32, tag="ps1", name="psv")
            for k in range(KT):
                nc.tensor.matmul(psv, wv_sb[:, k, mcol:mcol + P], xt_c[:, k, :],
                                 start=(k == 0), stop=(k == KT - 1))
            vts = vtp.tile([P, CH], F32, tag="vts", name="vts")
            nc.scalar.copy(vts, psv)
            for sb2 in range(CH // P):
                pst = ps1.tile([P, P], F32, tag="ps1", name="pst")
                nc.tensor.transpose(pst, vts[:, sb2 * P:(sb2 + 1) * P], id_sb)
                blk = c * (CH // P) + sb2
                nc.vector.tensor_copy(
                    VT[:, blk, 2 * p:2 * p + 2, 0:DK],
                    pst.rearrange("p (h d) -> p h d", h=2))

    def oproj_chunk(c):
        # part[s, :] = attn_cols @ W_o[:, cols].T for the 4 seq-blocks of chunk c
        for sb in range(CH // P):
            r0 = c * CH + sb * P
            ob = vtp.tile([P, 2, CH], F32, tag="ob", name="ob")
            for nt in range(2):
                po = ps1.tile([P, CH], F32, tag="ps1", name="po")
                for p2 in range(HP):
                    nc.tensor.matmul(
                        po,
                        AT[p2][c][:, sb * P:(sb + 1) * P],
                        wo_sb[:, p2, nt * CH:(nt + 1) * CH],
                        start=(p2 == 0), stop=(p2 == HP - 1))
                nc.vector.tensor_copy(ob[:, nt, :], po)
            nc.sync.dma_start(part_d[r0:r0 + P, :],
                              ob.rearrange("p a b -> p (a b)"))

    if PHASE == "projq":
        for p in range(HP):
            nc.sync.dma_start(part_d[p * P:(p + 1) * P, 0:CH], QT[p][0].bitcast(F32))
        return
    if PHASE == "proj":
        # keep QT/KT/VT live via a dummy output DMA
        for p in range(HP):
            nc.sync.dma_start(part_d[p * P:(p + 1) * P, 0:CH], QT[p][0].bitcast(F32))
            nc.sync.dma_start(part_d[p * P:(p + 1) * P, CH:2 * CH], KTt[p][0].bitcast(F32))
        nc.sync.dma_start(part_d[P:P + P, 0:NB * NH], VT[:, :, :, 0].bitcast(F32))
        return

    # ---- attention (scores transposed [k, q]; softmax w/o max-sub) ----
    # The two heads of a pair sit on partitions 0:64 / 64:128, so their K=64
    # scores matmuls land in different PE row-groups and overlap on HW.
    for qt in range(NCH):
        nkb = 4 * (qt + 1)          # causal: key blocks 0 .. 4*(qt+1)-1
        for p in range(HP):
            pv = [pvs.tile([DK + 1, CH], F32, tag="pv", name=f"pv{hh}")
                  for hh in range(2)]
            for g in range(nkb // 2):
                sp = [sps.tile([P, 2, CH], F32, tag="sp", name=f"sp{hh}")
                      for hh in range(2)]
                e = [ep.tile([P, 2, CH], F32R, tag="e", name=f"e{hh}")
                     for hh in range(2)]
                for j2 in range(2):
                    kb = 2 * g + j2
                    for hh in range(2):
                        off = hh * DK
                        nc.tensor.matmul(
                            sp[hh][:, j2, :],
                            KTt[p][kb // 4][off:off + DK, (kb % 4) * P:(kb % 4 + 1) * P],
                            QT[p][qt][off:off + DK, :],
                            start=True, stop=True)
                for hh in range(2):
                    nc.scalar.activation(e[hh], sp[hh], EXP, scale=0.125)
                dg = g - 2 * qt       # 0 or 1 for the two diagonal pair-groups
                for hh in range(2):
                    if dg >= 0:       # both kbs of this group are diagonal
                        nc.vector.tensor_mul(e[hh], e[hh], msk_sb[:, dg, :, :])
                    for j2 in range(2):
                        kb = 2 * g + j2
                        nc.tensor.matmul(
                            pv[hh][:, :],
                            VT[:, kb, 2 * p + hh, :],
                            e[hh][:, j2, :],
                            start=(kb == 0), stop=(kb == nkb - 1))
            for hh in range(2):
                off = hh * DK
                rec = rp.tile([1, CH], F32R, tag="rec", name="rec")
                with nc.allow_low_precision(reason="softmax recip fp32r"):
                    nc.vector.reciprocal(rec, pv[hh][DK:DK + 1, :])
                # Broadcast rec across 64 partitions via a K=1 matmul.
                rb = ps1.tile([DK, CH], F32, tag="ps1", name="rb")
                nc.tensor.matmul(rb, ones1, rec[:, :], start=True, stop=True)
                rbs = rp.tile([DK, CH], F32, tag="rbs", name="rbs")
                nc.vector.tensor_copy(rbs, rb)
                nc.vector.tensor_mul(AT[p][qt][off:off + DK, :], pv[hh][0:DK, :], rbs)
        # Output projection for chunk qt overlaps the next q-tile's attention.
        if PHASE != "attn":
            oproj_chunk(qt)
    if PHASE == "attn":
        for p in range(HP):
            for c in range(NCH):
                nc.sync.dma_start(part_d[(2 * c + p) * P:(2 * c + p + 1) * P, 0:CH],
                                  AT[p][c].bitcast(F32))


def _build(loop_n=0):
    nc = bacc.Bacc("TRN2", target_bir_lowering=False, debug=False)

    # All inputs pre-arranged host-side for contiguous per-partition DMA.
    xt_d = nc.dram_tensor("xt", [P, NCH, KT, CH], F32R, kind="ExternalInput")
    wqt_d = nc.dram_tensor("wqt", [P, KT, NH * DK], F32R, kind="ExternalInput")
    wkt_d = nc.dram_tensor("wkt", [P, KT, NH * DK], F32R, kind="ExternalInput")
    wvt_d = nc.dram_tensor("wvt", [P, KT, NH * DK], F32R, kind="ExternalInput")
    wot_d = nc.dram_tensor("wot", [P, HP, D], F32R, kind="ExternalInput")
    post_d = nc.dram_tensor("post", [P, S], F32, kind="ExternalInput")
    dmask_d = nc.dram_tensor("dmask", [P, 2, 2, CH], F32, kind="ExternalInput")
    ident_d = nc.dram_tensor("ident", [P, P], F32, kind="ExternalInput")
    part_d = nc.dram_tensor("part", [S, D], F32, kind="ExternalOutput")

    with tile.TileContext(nc) as tc:
        with tc.tile_pool(name="const", bufs=1) as const, \
             tc.tile_pool(name="xtp", bufs=2) as xtp, \
             tc.tile_pool(name="vtp", bufs=2) as vtp, \
             tc.tile_pool(name="ep", bufs=4) as ep, \
             tc.tile_pool(name="rp", bufs=2) as rp, \
             tc.tile_pool(name="ps1", bufs=2, space="PSUM") as ps1, \
             tc.tile_pool(name="sps", bufs=2, space="PSUM") as sps, \
             tc.tile_pool(name="pvs", bufs=2, space="PSUM") as pvs:

            wq_sb = const.tile([P, KT, NH * DK], F32R, tag="wq")
            wk_sb = const.tile([P, KT, NH * DK], F32R, tag="wk")
            wv_sb = const.tile([P, KT, NH * DK], F32R, tag="wv")
            wo_sb = const.tile([P, HP, D], F32R, tag="wo")
            pos_sb = const.tile([P, S], F32, tag="pos")
            ones1 = const.tile([1, DK], F32R, tag="ones1")
            msk_sb = const.tile([P, 2, 2, CH], F32, tag="msk")
            id_sb = const.tile([P, P], F32, tag="id")

            QT = [[const.tile([P, CH], F32R, tag=f"qt{p}_{c}", name=f"qt{p}_{c}")
                   for c in range(NCH)] for p in range(HP)]
            KTt = [[const.tile([P, CH], F32R, tag=f"kt{p}_{c}", name=f"kt{p}_{c}")
                    for c in range(NCH)] for p in range(HP)]
            AT = [[const.tile([P, CH], F32R, tag=f"at{p}_{c}", name=f"at{p}_{c}")
                   for c in range(NCH)] for p in range(HP)]
            # V natural: [s-part, block, head, 65]; col 64 = ones so the P@V
            # matmul also produces the softmax denominator row.
            VT = const.tile([P, NB, NH, DK + 1], F32R, tag="vt", name="vt")
            nc.vector.memset(ones1.bitcast(F32), 1.0)
            nc.vector.memset(VT[:, :, :, DK:DK + 1].bitcast(F32), 1.0)

            pools = (const, xtp, vtp, ep, rp, ps1, sps, pvs)
            tiles = (wq_sb, wk_sb, wv_sb, wo_sb, pos_sb, msk_sb, id_sb, ones1,
                     QT, KTt, AT, VT)
            dram = (xt_d, wqt_d, wkt_d, wvt_d, wot_d, post_d, dmask_d, ident_d,
                    part_d)

            if loop_n:
                with tc.For_i(0, loop_n, 1):
                    _emit(nc, tc, pools, tiles, dram)
            else:
                _emit(nc, tc, pools, tiles, dram)

    nc.compile()
    return nc


def _pos_embeddings():
    d_k = DK
    theta = (10000.0 ** (2.0 * np.arange(d_k, dtype=np.float32) / np.float32(d_k))
             ).astype(np.float32)
    pos = (np.arange(1, S, dtype=np.float32)[:, None] / theta).astype(np.float32)
    even = (np.arange(d_k) % 2 == 0)
    body = np.where(even[None, :], np.sin(pos), np.cos(pos)).astype(np.float32)
    return np.concatenate([np.zeros((1, d_k), np.float32), body], axis=0)  # [S, 64]


def make_in_maps(X, W_q, W_k, W_v, W_o):
    pos = _pos_embeddings()                       # [S, 64]
    post = np.ascontiguousarray(np.tile(pos.T, (2, 1)))   # [128, S]
    # dmask[kk, dg, j2, qq]: keep iff (2*dg + j2)*128 + kk <= qq
    kk = np.arange(P)[:, None, None, None]
    dg = np.arange(2)[None, :, None, None]
    j2 = np.arange(2)[None, None, :, None]
    qq = np.arange(CH)[None, None, None, :]
    dmask = (((2 * dg + j2) * P + kk) <= qq).astype(np.float32)
    ident = np.eye(P, dtype=np.float32)

    def pmajor(a2d, inner):     # [K*128, M] -> [128, K, M] partition-major
        Kn = a2d.shape[0] // P
        return np.ascontiguousarray(a2d.reshape(Kn, P, inner).transpose(1, 0, 2))

    in_maps = []
    for c in range(8):
        b, g = c // 4, c % 4
        rows = slice(256 * g, 256 * (g + 1))
        # xt[p, c, k, s'] = X[b][c*512+s', k*128+p]
        xt = np.ascontiguousarray(
            X[b].reshape(NCH, CH, KT, P).transpose(3, 0, 2, 1))
        in_maps.append({
            "xt": xt,
            "wqt": pmajor(W_q[rows, :].T.copy(), NH * DK),
            "wkt": pmajor(W_k[rows, :].T.copy(), NH * DK),
            "wvt": pmajor(W_v[rows, :].T.copy(), NH * DK),
            "wot": pmajor(W_o[:, rows].T.copy(), D),
            "post": post,
            "dmask": dmask,
            "ident": ident,
        })
    return in_maps


def kernel(X, W_q, W_k, W_v, W_o):
    X = np.ascontiguousarray(np.asarray(X, dtype=np.float32))
    W_q = np.asarray(W_q, dtype=np.float32)
    W_k = np.asarray(W_k, dtype=np.float32)
    W_v = np.asarray(W_v, dtype=np.float32)
    W_o = np.asarray(W_o, dtype=np.float32)

    if "nc" not in _CACHE:
        _CACHE["nc"] = _build()
    nc = _CACHE["nc"]

    in_maps = make_in_maps(X, W_q, W_k, W_v, W_o)
    res = run_bass_kernel_spmd(nc, in_maps, core_ids=list(range(8)))

    out = np.empty((2, S, D), np.float32)
    for b in range(2):
        acc = res.results[4 * b]["part"].astype(np.float32)
        for g in range(1, 4):
            acc = acc + res.results[4 * b + g]["part"]
        out[b] = acc + X[b]
    return out


# revision 24
# speedup vs baseline: 4.9113x; 4.9113x over previous
"""Trainium2 Bass kernel for a single decoder layer (MHA + residual).

Sharding: 8 cores = 2 batches x 4 head-groups. Core c handles batch c//4
and heads 4*(c%4) .. 4*(c%4)+3 (tensor-parallel over heads; the W_o
all-reduce is done host-side by summing the 4 partial outputs per batch).

Host passes inputs pre-arranged so every DMA reads large contiguous
per-partition blocks (measured ~10x faster than 2KB-strided patterns).

Hardcoded for: X [2, 2048, 1024], W_* [1024, 1024], 16 heads, d_k=64.
"""

import sys

for _p in ("/root/.axon_site/_ro/trn_rl_repo", "/opt/trn_rl_repo"):
    if _p not in sys.path:
        sys.path.append(_p)

import numpy as np

import concourse.bacc as bacc
import concourse.mybir as mybir
import concourse.tile as tile
from concourse.bass_utils import run_bass_kernel_spmd

P = 128
S = 2048          # per-core sequence length (one batch)
D = 1024
NH = 4            # heads per core
HP = 2            # head pairs per core
DK = 64
CH = 512          # seq chunk (free dim for projections / q tile)
NCH = S // CH     # 4
KT = D // P       # 8 contraction tiles for the projections
NB = S // P       # 16 key blocks
F32 = mybir.dt.float32
F32R = mybir.dt.float32r
EXP = mybir.ActivationFunctionType.Exp

_CACHE = {}


import os
PHASE = os.environ.get("KPHASE", "all")


def _emit(nc, tc, pools, tiles, dram):
    const, xtp, vtp, ep, rp, ps1, sps, pvs = pools
    (wq_sb, wk_sb, wv_sb, wo_sb, pos_sb, msk_sb, id_sb, ones1,
     QT, KTt, AT, VT) = tiles
    xt_d, wqt_d, wkt_d, wvt_d, wot_d, post_d, dmask_d, ident_d, part_d = dram

    xt_tiles = [xtp.tile([P, KT, CH], F32R, tag="xt", name=f"xt_c{c}")
                for c in range(NCH)]
    # chunk0 split in two so the first matmuls start after ~1MB of DMA
    nc.sync.dma_start(xt_tiles[0][:, 0:4, :], xt_d[:, 0, 0:4, :])
    nc.sync.dma_start(wq_sb, wqt_d[:, :, :])
    nc.sync.dma_start(xt_tiles[0][:, 4:8, :], xt_d[:, 0, 4:8, :])
    nc.sync.dma_start(wk_sb, wkt_d[:, :, :])
    nc.sync.dma_start(wv_sb, wvt_d[:, :, :])
    nc.sync.dma_start(pos_sb, post_d[:, :])
    nc.sync.dma_start(id_sb, ident_d[:, :])
    nc.sync.dma_start(msk_sb, dmask_d[:, :, :, :])
    nc.sync.dma_start(wo_sb, wot_d[:, :, :])

    if PHASE == "dmain":
        for c in range(NCH):
            if c > 0:
                nc.sync.dma_start(xt_tiles[c], xt_d[:, c, :, :])
            ob = vtp.tile([P, 2, CH], F32, tag="ob", name="ob")
            nc.vector.tensor_copy(ob[:, 0, :], xt_tiles[c][:, 0, :].bitcast(F32))
            nc.sync.dma_start(part_d[c * P:(c + 1) * P, 0:CH], ob[:, 0, :])
        return

    # ---- projections, chunk by chunk ----
    for c in range(NCH):
        xt_c = xt_tiles[c]
        if c > 0:
            nc.sync.dma_start(xt_c, xt_d[:, c, :, :])
        for p in range(HP if PHASE != "projq" else 1):
            mcol = p * P
            psq = ps1.tile([P, CH], F32, tag="ps1", name="psq")
            for k in range(KT):
                nc.tensor.matmul(psq, wq_sb[:, k, mcol:mcol + P], xt_c[:, k, :],
                                 start=(k == 0), stop=(k == KT - 1))
            nc.vector.tensor_add(QT[p][c], psq, pos_sb[:, c * CH:(c + 1) * CH])

            if PHASE == "projq":
                continue
            psk = ps1.tile([P, CH], F32, tag="ps1", name="psk")
            for k in range(KT):
                nc.tensor.matmul(psk, wk_sb[:, k, mcol:mcol + P], xt_c[:, k, :],
                                 start=(k == 0), stop=(k == KT - 1))
            nc.vector.tensor_add(KTt[p][c], psk, pos_sb[:, c * CH:(c + 1) * CH])

        # V directly in natural [s, head, dk] layout: one 8-matmul group per
        # 128-seq block with lhsT = X^T slice, rhs = all 4 heads of W_v^T.
        for sb2 in range(CH // P):
            psv = ps1.tile([P, NH * DK], F32, tag="ps1", name="psv")
            for k in range(KT):
                nc.tensor.matmul(psv, xt_c[:, k, sb2 * P:(sb2 + 1) * P],
                                 wv_sb[:, k, :],
                                 start=(k == 0), stop=(k == KT - 1))
            blk = c * (CH // P) + sb2
            nc.vector.tensor_copy(VT[:, blk, :, 0:DK],
                                  psv.rearrange("p (h d) -> p h d", h=NH))

    def oproj_chunk(c):
        # part[s, :] = attn_cols @ W_o[:, cols].T for the 4 seq-blocks of chunk c
        for sb in range(CH // P):
            r0 = c * CH + sb * P
            ob = vtp.tile([P, 2, CH], F32, tag="ob", name="ob")
            for nt in range(2):
                po = ps1.tile([P, CH], F32, tag="ps1", name="po")
                for p2 in range(HP):
                    nc.tensor.matmul(
                        po,
                        AT[p2][c][:, sb * P:(sb + 1) * P],
                        wo_sb[:, p2, nt * CH:(nt + 1) * CH],
                        start=(p2 == 0), stop=(p2 == HP - 1))
                nc.vector.tensor_copy(ob[:, nt, :], po)
            nc.sync.dma_start(part_d[r0:r0 + P, :],
                              ob.rearrange("p a b -> p (a b)"))

    if PHASE == "projq":
        for p in range(HP):
            nc.sync.dma_start(part_d[p * P:(p + 1) * P, 0:CH], QT[p][0].bitcast(F32))
        return
    if PHASE == "proj":
        # keep QT/KT/VT live via a dummy output DMA
        for p in range(HP):
            nc.sync.dma_start(part_d[p * P:(p + 1) * P, 0:CH], QT[p][0].bitcast(F32))
            nc.sync.dma_start(part_d[p * P:(p + 1) * P, CH:2 * CH], KTt[p][0].bitcast(F32))
        nc.sync.dma_start(part_d[P:P + P, 0:NB * NH], VT[:, :, :, 0].bitcast(F32))
        return

    # ---- attention (scores transposed [k, q]; softmax w/o max-sub) ----
    # The two heads of a pair sit on partitions 0:64 / 64:128, so their K=64
    # scores matmuls land in different PE row-groups and overlap on HW.
    for qt in range(NCH):
        nkb = 4 * (qt + 1)          # causal: key blocks 0 .. 4*(qt+1)-1
        for p in range(HP):
            pv = [pvs.tile([DK + 1, CH], F32, tag="pv", name=f"pv{hh}")
                  for hh in range(2)]
            for g in range(nkb // 2):
                sp = [sps.tile([P, 2, CH], F32, tag="sp", name=f"sp{hh}")
                      for hh in range(2)]
                e = [ep.tile([P, 2, CH], F32R, tag="e", name=f"e{hh}")
                     for hh in range(2)]
                for j2 in range(2):
                    kb = 2 * g + j2
                    for hh in range(2):
                        off = hh * DK
                        nc.tensor.matmul(
                            sp[hh][:, j2, :],
                            KTt[p][kb // 4][off:off + DK, (kb % 4) * P:(kb % 4 + 1) * P],
                            QT[p][qt][off:off + DK, :],
                            start=True, stop=True)
                for hh in range(2):
                    nc.scalar.activation(e[hh], sp[hh], EXP, scale=0.125)
                dg = g - 2 * qt       # 0 or 1 for the two diagonal pair-groups
                for hh in range(2):
                    if dg >= 0:       # both kbs of this group are diagonal
                        nc.vector.tensor_mul(e[hh], e[hh], msk_sb[:, dg, :, :])
                    for j2 in range(2):
                        kb = 2 * g + j2
                        nc.tensor.matmul(
                            pv[hh][:, :],
                            VT[:, kb, 2 * p + hh, :],
                            e[hh][:, j2, :],
                            start=(kb == 0), stop=(kb == nkb - 1))
            for hh in range(2):
                off = hh * DK
                rec = rp.tile([1, CH], F32R, tag="rec", name="rec")
                with nc.allow_low_precision(reason="softmax recip fp32r"):
                    nc.vector.reciprocal(rec, pv[hh][DK:DK + 1, :])
                # Broadcast rec across 64 partitions via a K=1 matmul.
                rb = ps1.tile([DK, CH], F32, tag="ps1", name="rb")
                nc.tensor.matmul(rb, ones1, rec[:, :], start=True, stop=True)
                rbs = rp.tile([DK, CH], F32, tag="rbs", name="rbs")
                nc.vector.tensor_copy(rbs, rb)
                nc.vector.tensor_mul(AT[p][qt][off:off + DK, :], pv[hh][0:DK, :], rbs)
        # Output projection for chunk qt overlaps the next q-tile's attention.
        if PHASE != "attn":
            oproj_chunk(qt)
    if PHASE == "attn":
        for p in range(HP):
            for c in range(NCH):
                nc.sync.dma_start(part_d[(2 * c + p) * P:(2 * c + p + 1) * P, 0:CH],
                                  AT[p][c].bitcast(F32))


def _build(loop_n=0):
    nc = bacc.Bacc("TRN2", target_bir_lowering=False, debug=False)

    # All inputs pre-arranged host-side for contiguous per-partition DMA.
    xt_d = nc.dram_tensor("xt", [P, NCH, KT, CH], F32R, kind="ExternalInput")
    wqt_d = nc.dram_tensor("wqt", [P, KT, NH * DK], F32R, kind="ExternalInput")
    wkt_d = nc.dram_tensor("wkt", [P, KT, NH * DK], F32R, kind="ExternalInput")
    wvt_d = nc.dram_tensor("wvt", [P, KT, NH * DK], F32R, kind="ExternalInput")
    wot_d = nc.dram_tensor("wot", [P, HP, D], F32R, kind="ExternalInput")
    post_d = nc.dram_tensor("post", [P, S], F32, kind="ExternalInput")
    dmask_d = nc.dram_tensor("dmask", [P, 2, 2, CH], F32, kind="ExternalInput")
    ident_d = nc.dram_tensor("ident", [P, P], F32, kind="ExternalInput")
    part_d = nc.dram_tensor("part", [S, D], F32, kind="ExternalOutput")

    with tile.TileContext(nc) as tc:
        with tc.tile_pool(name="const", bufs=1) as const, \
             tc.tile_pool(name="xtp", bufs=2) as xtp, \
             tc.tile_pool(name="vtp", bufs=2) as vtp, \
             tc.tile_pool(name="ep", bufs=4) as ep, \
             tc.tile_pool(name="rp", bufs=2) as rp, \
             tc.tile_pool(name="ps1", bufs=2, space="PSUM") as ps1, \
             tc.tile_pool(name="sps", bufs=2, space="PSUM") as sps, \
             tc.tile_pool(name="pvs", bufs=2, space="PSUM") as pvs:

            wq_sb = const.tile([P, KT, NH * DK], F32R, tag="wq")
            wk_sb = const.tile([P, KT, NH * DK], F32R, tag="wk")
            wv_sb = const.tile([P, KT, NH * DK], F32R, tag="wv")
            wo_sb = const.tile([P, HP, D], F32R, tag="wo")
            pos_sb = const.tile([P, S], F32, tag="pos")
            ones1 = const.tile([1, DK], F32R, tag="ones1")
            msk_sb = const.tile([P, 2, 2, CH], F32, tag="msk")
            id_sb = const.tile([P, P], F32, tag="id")

            QT = [[const.tile([P, CH], F32R, tag=f"qt{p}_{c}", name=f"qt{p}_{c}")
                   for c in range(NCH)] for p in range(HP)]
            KTt = [[const.tile([P, CH], F32R, tag=f"kt{p}_{c}", name=f"kt{p}_{c}")
                    for c in range(NCH)] for p in range(HP)]
            AT = [[const.tile([P, CH], F32R, tag=f"at{p}_{c}", name=f"at{p}_{c}")
                   for c in range(NCH)] for p in range(HP)]
            # V natural: [s-part, block, head, 65]; col 64 = ones so the P@V
            # matmul also produces the softmax denominator row.
            VT = const.tile([P, NB, NH, DK + 1], F32R, tag="vt", name="vt")
            nc.vector.memset(ones1.bitcast(F32), 1.0)
            nc.vector.memset(VT[:, :, :, DK:DK + 1].bitcast(F32), 1.0)

            pools = (const, xtp, vtp, ep, rp, ps1, sps, pvs)
            tiles = (wq_sb, wk_sb, wv_sb, wo_sb, pos_sb, msk_sb, id_sb, ones1,
                     QT, KTt, AT, VT)
            dram = (xt_d, wqt_d, wkt_d, wvt_d, wot_d, post_d, dmask_d, ident_d,
                    part_d)

            if loop_n:
                with tc.For_i(0, loop_n, 1):
                    _emit(nc, tc, pools, tiles, dram)
            else:
                _emit(nc, tc, pools, tiles, dram)

    nc.compile()
    return nc


def _pos_embeddings():
    d_k = DK
    theta = (10000.0 ** (2.0 * np.arange(d_k, dtype=np.float32) / np.float32(d_k))
             ).astype(np.float32)
    pos = (np.arange(1, S, dtype=np.float32)[:, None] / theta).astype(np.float32)
    even = (np.arange(d_k) % 2 == 0)
    body = np.where(even[None, :], np.sin(pos), np.cos(pos)).astype(np.float32)
    return np.concatenate([np.zeros((1, d_k), np.float32), body], axis=0)  # [S, 64]


def make_in_maps(X, W_q, W_k, W_v, W_o):
    pos = _pos_embeddings()                       # [S, 64]
    post = np.ascontiguousarray(np.tile(pos.T, (2, 1)))   # [128, S]
    # dmask[kk, dg, j2, qq]: keep iff (2*dg + j2)*128 + kk <= qq
    kk = np.arange(P)[:, None, None, None]
    dg = np.arange(2)[None, :, None, None]
    j2 = np.arange(2)[None, None, :, None]
    qq = np.arange(CH)[None, None, None, :]
    dmask = (((2 * dg + j2) * P + kk) <= qq).astype(np.float32)
    ident = np.eye(P, dtype=np.float32)

    def pmajor(a2d, inner):     # [K*128, M] -> [128, K, M] partition-major
        Kn = a2d.shape[0] // P
        return np.ascontiguousarray(a2d.reshape(Kn, P, inner).transpose(1, 0, 2))

    in_maps = []
    for c in range(8):
        b, g = c // 4, c % 4
        rows = slice(256 * g, 256 * (g + 1))
        # xt[p, c, k, s'] = X[b][c*512+s', k*128+p]
        xt = np.ascontiguousarray(
            X[b].reshape(NCH, CH, KT, P).transpose(3, 0, 2, 1))
        in_maps.append({
            "xt": xt,
            "wqt": pmajor(W_q[rows, :].T.copy(), NH * DK),
            "wkt": pmajor(W_k[rows, :].T.copy(), NH * DK),
            "wvt": pmajor(W_v[rows, :].T.copy(), NH * DK),
            "wot": pmajor(W_o[:, rows].T.copy(), D),
            "post": post,
            "dmask": dmask,
            "ident": ident,
        })
    return in_maps


def kernel(X, W_q, W_k, W_v, W_o):
    X = np.ascontiguousarray(np.asarray(X, dtype=np.float32))
    W_q = np.asarray(W_q, dtype=np.float32)
    W_k = np.asarray(W_k, dtype=np.float32)
    W_v = np.asarray(W_v, dtype=np.float32)
    W_o = np.asarray(W_o, dtype=np.float32)

    if "nc" not in _CACHE:
        _CACHE["nc"] = _build()
    nc = _CACHE["nc"]

    in_maps = make_in_maps(X, W_q, W_k, W_v, W_o)
    res = run_bass_kernel_spmd(nc, in_maps, core_ids=list(range(8)))

    out = np.empty((2, S, D), np.float32)
    for b in range(2):
        acc = res.results[4 * b]["part"].astype(np.float32)
        for g in range(1, 4):
            acc = acc + res.results[4 * b + g]["part"]
        out[b] = acc + X[b]
    return out
